# revision 4
# baseline (speedup 1.0000x reference)
"""Mixture-of-Experts (top-1 routing) Trainium2 kernel.

Strategy (expert-parallel, per sharding hint):
 - Router (softmax / argmax / top-prob) evaluated on host — 8192x8, i.e.
   0.002% of the FLOPs; its cost is dispatch bookkeeping.
 - Tokens are dispatched to the core owning their expert: core e receives
   the tokens routed to expert e (transposed, pre-scaled by the gate
   probability, padded to capacity C), plus W[e] and b[e].
 - Each core runs a dense [C,1024] @ [1024,1024] GEMM on the TensorEngine
   in float32r (full-rate fp32, ~1.5e-4 max rel err).  PSUM eviction fuses
   the bias: out = (bias * top_p) + psum in one DVE op per tile.
 - Host scatters the compact per-expert outputs back to token order
   (the "second all-to-all" / unshard step).
"""

import numpy as np

T, H, E = 8192, 1024, 8
N_CORES = 8
P = 128
KT = H // P          # 8 contraction tiles
NFREE = 512          # matmul moving free dim (one PSUM bank of fp32)
NT = H // NFREE      # 2 output column tiles

_BUILD_CACHE = {}


def _build(C):
    """Build the SPMD Bass module for per-core token capacity C (multiple of 128)."""
    import concourse.mybir as mybir
    import concourse.tile as tile
    from concourse import bacc

    MT = C // P
    DT = mybir.dt.float32r   # fp32 bits, full-rate matmul
    F32 = mybir.dt.float32
    ALU = mybir.AluOpType

    nc = bacc.Bacc("TRN2", target_bir_lowering=False, debug=False,
                   num_devices=N_CORES)

    xt_d = nc.dram_tensor("xt", [KT, P, C], DT, kind="ExternalInput").ap()
    w_d = nc.dram_tensor("w", [KT, P, H], DT, kind="ExternalInput").ap()
    bias_d = nc.dram_tensor("bias", [P, H], F32, kind="ExternalInput").ap()
    scale_d = nc.dram_tensor("scale", [MT, P], F32, kind="ExternalInput").ap()
    out_d = nc.dram_tensor("out", [MT, P, H], F32, kind="ExternalOutput").ap()

    CH = 3  # m-tiles per chunk (3m x 2n = 6 PSUM banks + 1 warmup bank)
    m_chunks = [list(range(s, min(s + CH, MT))) for s in range(0, MT, CH)]

    with tile.TileContext(nc) as tc:
        with (
            tc.tile_pool(name="ins", bufs=1) as ins,
            tc.tile_pool(name="psum", bufs=1, space="PSUM") as psum_pool,
            tc.tile_pool(name="outp", bufs=4) as outp,
        ):
            # xt tiles split by m-chunk so chunk 0's matmuls only wait on
            # their own slice of each k-tile.
            xt_sb = [[ins.tile([P, len(ch) * P], DT, name=f"xt{k}_{c}")
                      for c, ch in enumerate(m_chunks)] for k in range(KT)]
            w_sb = [ins.tile([P, H], DT, name=f"w{k}") for k in range(KT)]
            bias_sb = ins.tile([P, H], F32, name="bias")
            scale_sb = ins.tile([P, MT], F32, name="scale")

            # PE warm-up: ~8 dummy matmuls on a zeroed tile run during the
            # DMA head phase so the HAM clock-gate opens (1.2->2.4 GHz)
            # before the first real matmul issues.
            wz = ins.tile([P, P + NFREE], F32, name="wz")
            nc.gpsimd.memset(wz[:], 0)
            warm_ps = psum_pool.tile([P, NFREE], F32, name="wps")
            for _ in range(8):
                nc.tensor.matmul(warm_ps[:],
                                 wz[:, :P].bitcast(DT), wz[:, P:].bitcast(DT),
                                 start=True, stop=True)

            # small control inputs ride the (slower) SWDGE GpSimd queue,
            # keeping the two HWDGE queues free for the xt / w streams
            nc.gpsimd.dma_start(scale_sb[:], scale_d.rearrange("m p -> p m"))
            nc.gpsimd.dma_start(bias_sb[:], bias_d[:])
            # chunk-0 slices + all of w first, k-major, on parallel queues
            for k in range(KT):
                nc.sync.dma_start(xt_sb[k][0][:],
                                  xt_d[k][:, 0:len(m_chunks[0]) * P])
                nc.scalar.dma_start(w_sb[k][:], w_d[k])
            for c in range(1, len(m_chunks)):
                lo = m_chunks[c][0] * P
                hi = (m_chunks[c][-1] + 1) * P
                for k in range(KT):
                    eng = nc.sync if k % 2 == 0 else nc.scalar
                    eng.dma_start(xt_sb[k][c][:], xt_d[k][:, lo:hi])

            for c, chunk in enumerate(m_chunks):
                ps = {}
                for ci, m in enumerate(chunk):
                    for n in range(NT):
                        ps[m, n] = psum_pool.tile([P, NFREE], F32,
                                                  name=f"ps{ci}_{n}")
                for k in range(KT):
                    for ci, m in enumerate(chunk):
                        for n in range(NT):
                            nc.tensor.matmul(
                                ps[m, n][:],
                                xt_sb[k][c][:, ci * P:(ci + 1) * P],
                                w_sb[k][:, n * NFREE:(n + 1) * NFREE],
                                start=(k == 0), stop=(k == KT - 1),
                            )
                for mi, m in enumerate(chunk):
                    t = outp.tile([P, H], F32, name="osb")
                    for n in range(NT):
                        nsl = slice(n * NFREE, (n + 1) * NFREE)
                        # out = bias * top_p + psum   (single DVE op)
                        nc.vector.scalar_tensor_tensor(
                            t[:, nsl], bias_sb[:, nsl],
                            scale_sb[:, m:m + 1], ps[m, n][:],
                            op0=ALU.mult, op1=ALU.add,
                        )
                    eng = nc.sync if mi % 2 == 0 else nc.scalar
                    eng.dma_start(out_d[m], t[:])

    nc.compile()
    return nc


def kernel(input, gate, W, b):
    from concourse import bass_utils

    input = np.ascontiguousarray(input, dtype=np.float32)
    gate = np.ascontiguousarray(gate, dtype=np.float32)
    W = np.ascontiguousarray(W, dtype=np.float32)
    b = np.ascontiguousarray(b, dtype=np.float32)

    # ---- router (host): top-1 expert + its softmax probability ----
    g = gate.astype(np.float64)
    gm = g.max(axis=1, keepdims=True)
    top_p = (1.0 / np.exp(g - gm).sum(axis=1)).astype(np.float32)
    e_t = np.argmax(gate, axis=1)

    counts = np.bincount(e_t, minlength=E)
    order = np.argsort(e_t, kind="stable")
    starts = np.zeros(E + 1, dtype=np.int64)
    np.cumsum(counts, out=starts[1:])

    C = max(P, int(-(-counts.max() // P)) * P)
    MT = C // P

    if C not in _BUILD_CACHE:
        _BUILD_CACHE[C] = _build(C)
    nc = _BUILD_CACHE[C]

    in_maps = []
    ids_per_e = []
    for e in range(E):
        ids = order[starts[e]:starts[e + 1]]
        ids_per_e.append(ids)
        n_e = len(ids)

        xt = np.zeros((KT, P, C), dtype=np.float32)
        # tokens pre-scaled by their gate probability
        xt.reshape(H, C)[:, :n_e] = input[ids].T * top_p[ids][None, :]

        scale = np.zeros((MT, P), dtype=np.float32)
        scale.reshape(C)[:n_e] = top_p[ids]

        in_maps.append({
            "xt": xt,
            "w": W[e].reshape(KT, P, H),
            "bias": np.ascontiguousarray(np.broadcast_to(b[e], (P, H))),
            "scale": scale,
        })

    res = bass_utils.run_bass_kernel_spmd(nc, in_maps,
                                          core_ids=list(range(N_CORES)))

    out = np.empty((T, H), dtype=np.float32)
    for e in range(E):
        ids = ids_per_e[e]
        out[ids] = res.results[e]["out"].reshape(C, H)[:len(ids)]
    return out


# revision 5
# speedup vs baseline: 1.0504x; 1.0504x over previous
"""Mixture-of-Experts (top-1 routing) Trainium2 kernel.

Strategy (expert-parallel, per sharding hint):
 - Router (softmax / argmax / top-prob) evaluated on host — 8192x8, i.e.
   0.002% of the FLOPs; its cost is dispatch bookkeeping.
 - Tokens are dispatched to the core owning their expert: core e receives
   the tokens routed to expert e (transposed, pre-scaled by the gate
   probability, padded to capacity C), plus W[e] and b[e].
 - Each core runs a dense [C,1024] @ [1024,1024] GEMM on the TensorEngine
   in float32r (full-rate fp32, ~1.5e-4 max rel err).  PSUM eviction fuses
   the bias: out = (bias * top_p) + psum in one DVE op per tile.
 - Host scatters the compact per-expert outputs back to token order
   (the "second all-to-all" / unshard step).
"""

import numpy as np

T, H, E = 8192, 1024, 8
N_CORES = 8
P = 128
KT = H // P          # 8 contraction tiles
NFREE = 512          # matmul moving free dim (one PSUM bank of fp32)
NT = H // NFREE      # 2 output column tiles

_BUILD_CACHE = {}


def _build(C):
    """Build the SPMD Bass module for per-core token capacity C (multiple of 128)."""
    import concourse.mybir as mybir
    import concourse.tile as tile
    from concourse import bacc

    MT = C // P
    DT = mybir.dt.float32r   # fp32 bits, full-rate matmul
    F32 = mybir.dt.float32
    ALU = mybir.AluOpType

    nc = bacc.Bacc("TRN2", target_bir_lowering=False, debug=False,
                   num_devices=N_CORES)

    xt_d = nc.dram_tensor("xt", [KT, P, C], DT, kind="ExternalInput").ap()
    w_d = nc.dram_tensor("w", [KT, P, H], DT, kind="ExternalInput").ap()
    bias_d = nc.dram_tensor("bias", [P, H], F32, kind="ExternalInput").ap()
    scale_d = nc.dram_tensor("scale", [P, MT], F32, kind="ExternalInput").ap()
    out_d = nc.dram_tensor("out", [MT, P, H], F32, kind="ExternalOutput").ap()

    CH = 4  # m-tiles per chunk (4m x 2n = 8 PSUM banks)
    m_chunks = [list(range(s, min(s + CH, MT))) for s in range(0, MT, CH)]

    with tile.TileContext(nc) as tc:
        with (
            tc.tile_pool(name="ins", bufs=1) as ins,
            tc.tile_pool(name="psum", bufs=1, space="PSUM") as psum_pool,
            tc.tile_pool(name="outp", bufs=4) as outp,
        ):
            xt_sb = [ins.tile([P, C], DT, name=f"xt{k}") for k in range(KT)]
            w_sb = [ins.tile([P, H], DT, name=f"w{k}") for k in range(KT)]
            bias_sb = ins.tile([P, H], F32, name="bias")
            scale_sb = ins.tile([P, MT], F32, name="scale")

            # PE warm-up: 8 dummy matmuls on a zeroed tile run during the
            # DMA head phase so the HAM clock-gate opens (1.2->2.4 GHz)
            # before the first real matmul issues.  The warm-up PSUM tile
            # shares the ps0_0 slot: its last write completes long before
            # chunk 0's data arrives, so the WAW hand-off is free.
            wz = ins.tile([P, P + NFREE], F32, name="wz")
            nc.gpsimd.memset(wz[:], 0)
            warm_ps = psum_pool.tile([P, NFREE], F32, name="ps0_0")
            for _ in range(8):
                nc.tensor.matmul(warm_ps[:],
                                 wz[:, :P].bitcast(DT), wz[:, P:].bitcast(DT),
                                 start=True, stop=True)

            # k-major stream of xt (Sync queue) / w (Scalar queue); the tiny
            # scale / bias inputs are slotted in after the first k-pair.
            for k in range(KT):
                nc.sync.dma_start(xt_sb[k][:], xt_d[k])
                nc.scalar.dma_start(w_sb[k][:], w_d[k])
                if k == 0:
                    nc.sync.dma_start(scale_sb[:], scale_d[:])
                if k == 1:
                    nc.scalar.dma_start(bias_sb[:], bias_d[:])

            for chunk in m_chunks:
                ps = {}
                for ci, m in enumerate(chunk):
                    for n in range(NT):
                        ps[m, n] = psum_pool.tile([P, NFREE], F32,
                                                  name=f"ps{ci}_{n}")
                for k in range(KT):
                    for ci, m in enumerate(chunk):
                        for n in range(NT):
                            nc.tensor.matmul(
                                ps[m, n][:],
                                xt_sb[k][:, m * P:(m + 1) * P],
                                w_sb[k][:, n * NFREE:(n + 1) * NFREE],
                                start=(k == 0), stop=(k == KT - 1),
                            )
                for mi, m in enumerate(chunk):
                    t = outp.tile([P, H], F32, name="osb")
                    for n in range(NT):
                        nsl = slice(n * NFREE, (n + 1) * NFREE)
                        # out = bias * top_p + psum   (single DVE op)
                        nc.vector.scalar_tensor_tensor(
                            t[:, nsl], bias_sb[:, nsl],
                            scale_sb[:, m:m + 1], ps[m, n][:],
                            op0=ALU.mult, op1=ALU.add,
                        )
                    eng = nc.sync if mi % 2 == 0 else nc.scalar
                    eng.dma_start(out_d[m], t[:])

    nc.compile()
    return nc


def kernel(input, gate, W, b):
    from concourse import bass_utils

    input = np.ascontiguousarray(input, dtype=np.float32)
    gate = np.ascontiguousarray(gate, dtype=np.float32)
    W = np.ascontiguousarray(W, dtype=np.float32)
    b = np.ascontiguousarray(b, dtype=np.float32)

    # ---- router (host): top-1 expert + its softmax probability ----
    g = gate.astype(np.float64)
    gm = g.max(axis=1, keepdims=True)
    top_p = (1.0 / np.exp(g - gm).sum(axis=1)).astype(np.float32)
    e_t = np.argmax(gate, axis=1)

    counts = np.bincount(e_t, minlength=E)
    order = np.argsort(e_t, kind="stable")
    starts = np.zeros(E + 1, dtype=np.int64)
    np.cumsum(counts, out=starts[1:])

    C = max(P, int(-(-counts.max() // P)) * P)
    MT = C // P

    if C not in _BUILD_CACHE:
        _BUILD_CACHE[C] = _build(C)
    nc = _BUILD_CACHE[C]

    in_maps = []
    ids_per_e = []
    for e in range(E):
        ids = order[starts[e]:starts[e + 1]]
        ids_per_e.append(ids)
        n_e = len(ids)

        xt = np.zeros((KT, P, C), dtype=np.float32)
        # tokens pre-scaled by their gate probability
        xt.reshape(H, C)[:, :n_e] = input[ids].T * top_p[ids][None, :]

        scale = np.zeros((MT, P), dtype=np.float32)
        scale.reshape(C)[:n_e] = top_p[ids]
        scale = np.ascontiguousarray(scale.T)

        in_maps.append({
            "xt": xt,
            "w": W[e].reshape(KT, P, H),
            "bias": np.ascontiguousarray(np.broadcast_to(b[e], (P, H))),
            "scale": scale,
        })

    res = bass_utils.run_bass_kernel_spmd(nc, in_maps,
                                          core_ids=list(range(N_CORES)))

    out = np.empty((T, H), dtype=np.float32)
    for e in range(E):
        ids = ids_per_e[e]
        out[ids] = res.results[e]["out"].reshape(C, H)[:len(ids)]
    return out


# revision 6
# speedup vs baseline: 1.2751x; 1.2139x over previous
"""Mixture-of-Experts (top-1 routing) Trainium2 kernel.

Strategy (expert-parallel, per sharding hint):
 - Router (softmax / argmax / top-prob) evaluated on host — 8192x8, i.e.
   0.002% of the FLOPs; its cost is dispatch bookkeeping.
 - Tokens are dispatched to the core owning their expert: core e receives
   the tokens routed to expert e (transposed, pre-scaled by the gate
   probability, padded to capacity C), plus W[e] and b[e].
 - Each core runs a dense [C,1024] @ [1024,1024] GEMM on the TensorEngine
   in float32r (full-rate fp32, ~1.5e-4 max rel err).  PSUM eviction fuses
   the bias: out = (bias * top_p) + psum in one DVE op per tile.
 - Host scatters the compact per-expert outputs back to token order
   (the "second all-to-all" / unshard step).
"""

import numpy as np

T, H, E = 8192, 1024, 8
N_CORES = 8
P = 128
KT = H // P          # 8 contraction tiles
NFREE = 512          # matmul moving free dim (one PSUM bank of fp32)
NT = H // NFREE      # 2 output column tiles

_BUILD_CACHE = {}


def _build(C):
    """Build the SPMD Bass module for per-core token capacity C (multiple of 128)."""
    import concourse.mybir as mybir
    import concourse.tile as tile
    from concourse import bacc

    MT = C // P
    DT = mybir.dt.float16    # half-precision I/O, full-rate matmul
    F32 = mybir.dt.float32
    F16 = mybir.dt.float16
    ALU = mybir.AluOpType

    nc = bacc.Bacc("TRN2", target_bir_lowering=False, debug=False,
                   num_devices=N_CORES)

    xt_d = nc.dram_tensor("xt", [KT, P, C], DT, kind="ExternalInput").ap()
    w_d = nc.dram_tensor("w", [KT, P, H], DT, kind="ExternalInput").ap()
    bias_d = nc.dram_tensor("bias", [P, H], F16, kind="ExternalInput").ap()
    scale_d = nc.dram_tensor("scale", [P, MT], F32, kind="ExternalInput").ap()
    out_d = nc.dram_tensor("out", [MT, P, H], F16, kind="ExternalOutput").ap()

    CH = 4  # m-tiles per chunk (4m x 2n = 8 PSUM banks)
    m_chunks = [list(range(s, min(s + CH, MT))) for s in range(0, MT, CH)]
    # split the final chunk into single m-tiles so the kernel tail
    # (eviction + out-DMA after the last matmul) is as short as possible
    if len(m_chunks) > 1 and len(m_chunks[-1]) > 1:
        last = m_chunks.pop()
        m_chunks.extend([m] for m in last)

    with tile.TileContext(nc) as tc:
        with (
            tc.tile_pool(name="ins", bufs=1) as ins,
            tc.tile_pool(name="psum", bufs=1, space="PSUM") as psum_pool,
            tc.tile_pool(name="outp", bufs=4) as outp,
        ):
            xt_sb = [ins.tile([P, C], DT, name=f"xt{k}") for k in range(KT)]
            w_sb = [ins.tile([P, H], DT, name=f"w{k}") for k in range(KT)]
            bias_sb = ins.tile([P, H], F16, name="bias")
            scale_sb = ins.tile([P, MT], F32, name="scale")

            # PE warm-up: 8 dummy matmuls on a zeroed tile run during the
            # DMA head phase so the HAM clock-gate opens (1.2->2.4 GHz)
            # before the first real matmul issues.  The warm-up PSUM tile
            # shares the ps0_0 slot: its last write completes long before
            # chunk 0's data arrives, so the WAW hand-off is free.
            wz = ins.tile([P, P + NFREE], DT, name="wz")
            nc.gpsimd.memset(wz[:], 0)
            warm_ps = psum_pool.tile([P, NFREE], F32, name="ps0_0")
            for _ in range(8):
                nc.tensor.matmul(warm_ps[:], wz[:, :P], wz[:, P:],
                                 start=True, stop=True)

            # k-major stream of xt (Sync queue) / w (Scalar queue); the tiny
            # scale / bias inputs are slotted in after the first k-pair.
            for k in range(KT):
                nc.sync.dma_start(xt_sb[k][:], xt_d[k])
                nc.scalar.dma_start(w_sb[k][:], w_d[k])
                if k == 0:
                    nc.sync.dma_start(scale_sb[:], scale_d[:])
                if k == 1:
                    nc.scalar.dma_start(bias_sb[:], bias_d[:])

            for chunk in m_chunks:
                ps = {}
                for ci, m in enumerate(chunk):
                    for n in range(NT):
                        ps[m, n] = psum_pool.tile([P, NFREE], F32,
                                                  name=f"ps{ci}_{n}")
                for k in range(KT):
                    for ci, m in enumerate(chunk):
                        for n in range(NT):
                            nc.tensor.matmul(
                                ps[m, n][:],
                                xt_sb[k][:, m * P:(m + 1) * P],
                                w_sb[k][:, n * NFREE:(n + 1) * NFREE],
                                start=(k == 0), stop=(k == KT - 1),
                            )
                for mi, m in enumerate(chunk):
                    t = outp.tile([P, H], F16, name="osb")
                    for n in range(NT):
                        nsl = slice(n * NFREE, (n + 1) * NFREE)
                        # out = bias * top_p + psum   (single DVE op)
                        nc.vector.scalar_tensor_tensor(
                            t[:, nsl], bias_sb[:, nsl],
                            scale_sb[:, m:m + 1], ps[m, n][:],
                            op0=ALU.mult, op1=ALU.add,
                        )
                    eng = nc.sync if mi % 2 == 0 else nc.scalar
                    eng.dma_start(out_d[m], t[:])

    nc.compile()
    return nc


def kernel(input, gate, W, b):
    from concourse import bass_utils

    input = np.ascontiguousarray(input, dtype=np.float32)
    gate = np.ascontiguousarray(gate, dtype=np.float32)
    W = np.ascontiguousarray(W, dtype=np.float32)
    b = np.ascontiguousarray(b, dtype=np.float32)

    # ---- router (host): top-1 expert + its softmax probability ----
    g = gate.astype(np.float64)
    gm = g.max(axis=1, keepdims=True)
    top_p = (1.0 / np.exp(g - gm).sum(axis=1)).astype(np.float32)
    e_t = np.argmax(gate, axis=1)

    counts = np.bincount(e_t, minlength=E)
    order = np.argsort(e_t, kind="stable")
    starts = np.zeros(E + 1, dtype=np.int64)
    np.cumsum(counts, out=starts[1:])

    C = max(P, int(-(-counts.max() // P)) * P)
    MT = C // P

    if C not in _BUILD_CACHE:
        _BUILD_CACHE[C] = _build(C)
    nc = _BUILD_CACHE[C]

    in_maps = []
    ids_per_e = []
    for e in range(E):
        ids = order[starts[e]:starts[e + 1]]
        ids_per_e.append(ids)
        n_e = len(ids)

        xt = np.zeros((KT, P, C), dtype=np.float16)
        # tokens pre-scaled by their gate probability
        xt.reshape(H, C)[:, :n_e] = (input[ids].T * top_p[ids][None, :]).astype(np.float16)

        scale = np.zeros((MT, P), dtype=np.float32)
        scale.reshape(C)[:n_e] = top_p[ids]
        scale = np.ascontiguousarray(scale.T)

        in_maps.append({
            "xt": xt,
            "w": W[e].astype(np.float16).reshape(KT, P, H),
            "bias": np.ascontiguousarray(np.broadcast_to(b[e].astype(np.float16), (P, H))),
            "scale": scale,
        })

    res = bass_utils.run_bass_kernel_spmd(nc, in_maps,
                                          core_ids=list(range(N_CORES)))

    out = np.empty((T, H), dtype=np.float32)
    for e in range(E):
        ids = ids_per_e[e]
        out[ids] = res.results[e]["out"].reshape(C, H)[:len(ids)].astype(np.float32)
    return out


# revision 7
# speedup vs baseline: 1.2984x; 1.0183x over previous
"""Mixture-of-Experts (top-1 routing) Trainium2 kernel.

Strategy (expert-parallel, per sharding hint):
 - Router (softmax / argmax / top-prob) evaluated on host — 8192x8, i.e.
   0.002% of the FLOPs; its cost is dispatch bookkeeping.
 - Tokens are dispatched to the core owning their expert: core e receives
   the tokens routed to expert e (transposed, pre-scaled by the gate
   probability, padded to capacity C), plus W[e] and b[e].
 - Each core runs a dense [C,1024] @ [1024,1024] GEMM on the TensorEngine
   in float32r (full-rate fp32, ~1.5e-4 max rel err).  PSUM eviction fuses
   the bias: out = (bias * top_p) + psum in one DVE op per tile.
 - Host scatters the compact per-expert outputs back to token order
   (the "second all-to-all" / unshard step).
"""

import numpy as np

T, H, E = 8192, 1024, 8
N_CORES = 8
P = 128
KT = H // P          # 8 contraction tiles
NFREE = 512          # matmul moving free dim (one PSUM bank of fp32)
NT = H // NFREE      # 2 output column tiles

_BUILD_CACHE = {}


def _build(C):
    """Build the SPMD Bass module for per-core token capacity C (multiple of 128)."""
    import concourse.mybir as mybir
    import concourse.tile as tile
    from concourse import bacc

    MT = C // P
    DT = mybir.dt.float16    # half-precision I/O, full-rate matmul
    F32 = mybir.dt.float32
    F16 = mybir.dt.float16
    ALU = mybir.AluOpType

    nc = bacc.Bacc("TRN2", target_bir_lowering=False, debug=False,
                   num_devices=N_CORES)

    xt_d = nc.dram_tensor("xt", [KT, P, C], DT, kind="ExternalInput").ap()
    w_d = nc.dram_tensor("w", [KT, P, H], DT, kind="ExternalInput").ap()
    bias_d = nc.dram_tensor("bias", [P, H], F16, kind="ExternalInput").ap()
    scale_d = nc.dram_tensor("scale", [P, MT], F32, kind="ExternalInput").ap()
    out_d = nc.dram_tensor("out", [MT, P, H], F16, kind="ExternalOutput").ap()

    CH = 4  # m-tiles per chunk (4m x 2n = 8 PSUM banks)
    m_chunks = [list(range(s, min(s + CH, MT))) for s in range(0, MT, CH)]
    # split the final chunk into single m-tiles so the kernel tail
    # (eviction + out-DMA after the last matmul) is as short as possible
    if len(m_chunks) > 1 and len(m_chunks[-1]) > 1:
        last = m_chunks.pop()
        m_chunks.extend([m] for m in last)

    with tile.TileContext(nc) as tc:
        with (
            tc.tile_pool(name="ins", bufs=1) as ins,
            tc.tile_pool(name="psum", bufs=1, space="PSUM") as psum_pool,
            tc.tile_pool(name="outp", bufs=4) as outp,
        ):
            xt_sb = [ins.tile([P, C], DT, name=f"xt{k}") for k in range(KT)]
            w_sb = [ins.tile([P, H], DT, name=f"w{k}") for k in range(KT)]
            bias_sb = ins.tile([P, H], F16, name="bias")
            scale_sb = ins.tile([P, MT], F32, name="scale")

            # PE warm-up: 8 dummy matmuls on a zeroed tile run during the
            # DMA head phase so the HAM clock-gate opens (1.2->2.4 GHz)
            # before the first real matmul issues.  The warm-up PSUM tile
            # shares the ps0_0 slot: its last write completes long before
            # chunk 0's data arrives, so the WAW hand-off is free.
            wz = ins.tile([P, P + NFREE], DT, name="wz")
            nc.gpsimd.memset(wz[:], 0)
            warm_ps = psum_pool.tile([P, NFREE], F32, name="ps0_0")
            for _ in range(8):
                nc.tensor.matmul(warm_ps[:], wz[:, :P], wz[:, P:],
                                 start=True, stop=True)

            # tiny scale/bias inputs ride the SWDGE GpSimd queue, keeping
            # both HWDGE queues free for the k-major xt / w streams
            nc.gpsimd.dma_start(scale_sb[:], scale_d[:])
            nc.gpsimd.dma_start(bias_sb[:], bias_d[:])
            for k in range(KT):
                nc.sync.dma_start(xt_sb[k][:], xt_d[k])
                nc.scalar.dma_start(w_sb[k][:], w_d[k])

            for chunk in m_chunks:
                ps = {}
                for m in chunk:
                    for n in range(NT):
                        ps[m, n] = psum_pool.tile([P, NFREE], F32,
                                                  name=f"ps{m % CH}_{n}")
                for k in range(KT):
                    for m in chunk:
                        for n in range(NT):
                            nc.tensor.matmul(
                                ps[m, n][:],
                                xt_sb[k][:, m * P:(m + 1) * P],
                                w_sb[k][:, n * NFREE:(n + 1) * NFREE],
                                start=(k == 0), stop=(k == KT - 1),
                            )
                for mi, m in enumerate(chunk):
                    t = outp.tile([P, H], F16, name="osb")
                    for n in range(NT):
                        nsl = slice(n * NFREE, (n + 1) * NFREE)
                        # out = bias * top_p + psum   (single DVE op)
                        nc.vector.scalar_tensor_tensor(
                            t[:, nsl], bias_sb[:, nsl],
                            scale_sb[:, m:m + 1], ps[m, n][:],
                            op0=ALU.mult, op1=ALU.add,
                        )
                        if len(chunk) == 1:
                            # tail chunks: ship each half as soon as its
                            # eviction lands
                            eng = nc.sync if n == 0 else nc.scalar
                            eng.dma_start(out_d[m][:, nsl], t[:, nsl])
                    if len(chunk) > 1:
                        eng = nc.sync if mi % 2 == 0 else nc.scalar
                        eng.dma_start(out_d[m], t[:])

    nc.compile()
    return nc


def kernel(input, gate, W, b):
    from concourse import bass_utils

    input = np.ascontiguousarray(input, dtype=np.float32)
    gate = np.ascontiguousarray(gate, dtype=np.float32)
    W = np.ascontiguousarray(W, dtype=np.float32)
    b = np.ascontiguousarray(b, dtype=np.float32)

    # ---- router (host): top-1 expert + its softmax probability ----
    g = gate.astype(np.float64)
    gm = g.max(axis=1, keepdims=True)
    top_p = (1.0 / np.exp(g - gm).sum(axis=1)).astype(np.float32)
    e_t = np.argmax(gate, axis=1)

    counts = np.bincount(e_t, minlength=E)
    order = np.argsort(e_t, kind="stable")
    starts = np.zeros(E + 1, dtype=np.int64)
    np.cumsum(counts, out=starts[1:])

    C = max(P, int(-(-counts.max() // P)) * P)
    MT = C // P

    if C not in _BUILD_CACHE:
        _BUILD_CACHE[C] = _build(C)
    nc = _BUILD_CACHE[C]

    in_maps = []
    ids_per_e = []
    for e in range(E):
        ids = order[starts[e]:starts[e + 1]]
        ids_per_e.append(ids)
        n_e = len(ids)

        xt = np.zeros((KT, P, C), dtype=np.float16)
        # tokens pre-scaled by their gate probability
        xt.reshape(H, C)[:, :n_e] = (input[ids].T * top_p[ids][None, :]).astype(np.float16)

        scale = np.zeros((MT, P), dtype=np.float32)
        scale.reshape(C)[:n_e] = top_p[ids]
        scale = np.ascontiguousarray(scale.T)

        in_maps.append({
            "xt": xt,
            "w": W[e].astype(np.float16).reshape(KT, P, H),
            "bias": np.ascontiguousarray(np.broadcast_to(b[e].astype(np.float16), (P, H))),
            "scale": scale,
        })

    res = bass_utils.run_bass_kernel_spmd(nc, in_maps,
                                          core_ids=list(range(N_CORES)))

    out = np.empty((T, H), dtype=np.float32)
    for e in range(E):
        ids = ids_per_e[e]
        out[ids] = res.results[e]["out"].reshape(C, H)[:len(ids)].astype(np.float32)
    return out
